# revision 11
# baseline (speedup 1.0000x reference)
"""CQVAE loss kernel for Trainium2, data-parallel over batch on 8 NeuronCores.

loss = kld(qy) + mse(gather(rzs), zs[:, :Sg]) + bias(best, best_gt)
       + bias(gather(pts), gts)
where bias(p, g) = mse(p, g) + 10 * mse(p[..., MARK, :], g[..., MARK, :]).

Each core handles 16 of the 128 batches: the mapping-gathers run on-device
via indirect DMA (one row per partition), squared-difference sums are
reduced per partition on the vector/scalar engines, and a ones-matmul
folds partitions.  Per-core partial sums (8 floats) are combined on host.
"""

import sys

import numpy as np

try:
    import concourse  # noqa: F401
except ImportError:  # pragma: no cover
    sys.path.insert(0, "/opt/trn_rl_repo")

import concourse.bass as bass
import concourse.mybir as mybir
import concourse.tile as tile
from concourse import bacc
from concourse.bass_utils import run_bass_kernel_spmd

F32 = mybir.dt.float32
I32 = mybir.dt.int32
AX = mybir.AxisListType
OP = mybir.AluOpType
ACTF = mybir.ActivationFunctionType

NCORES = 8
B, S, SG, D, P, V = 128, 256, 128, 1024, 118, 64
BL = B // NCORES  # batches per core
P2 = 2 * P  # 236 floats per point-row
MARK = (0, 29, 88, 117)
EPS = 1e-20
ALPHA = 10.0

KB = 8  # gts/pts batches per bias group
NSTAT = 32
AE0 = 16  # stats columns 16.. hold per-piece ae accumulators

_module = None
last_results = None  # BassKernelResults of the most recent run (for profiling)

# The NEFF exit epilogue emits per-semaphore completion waits for the whole
# declared kernel semaphore range (~7us for 250 sems).  This kernel fits in
# a smaller range, so narrow it before building the module.
_SEM_CAP = 176
_orig_sem_range = bass.get_kernel_semaphore_range


def _capped_sem_range():
    r = _orig_sem_range()
    return range(r.start, min(r.stop, _SEM_CAP))


def _build_module():
    bass.get_kernel_semaphore_range = _capped_sem_range
    try:
        nc = bacc.Bacc()
    finally:
        bass.get_kernel_semaphore_range = _orig_sem_range

    zs = nc.dram_tensor("zs", [BL * S, D], F32, kind="ExternalInput")
    rzs = nc.dram_tensor("rzs", [BL * S, D], F32, kind="ExternalInput")
    pts = nc.dram_tensor("pts", [BL * S, P2], F32, kind="ExternalInput")
    gts = nc.dram_tensor("gts", [BL * SG, P2], F32, kind="ExternalInput")
    qy = nc.dram_tensor("qy", [BL * S, V], F32, kind="ExternalInput")
    best = nc.dram_tensor("best", [B, P2], F32, kind="ExternalInput")
    best_gt = nc.dram_tensor("best_gt", [B, P2], F32, kind="ExternalInput")
    # idx[i, b] = b*S + mapping[b, i]: flat row into the per-core rzs/pts shard
    idx = nc.dram_tensor("idx", [SG, BL], I32, kind="ExternalInput")
    out = nc.dram_tensor("out", [1, NSTAT], F32, kind="ExternalOutput")

    QCOLS = BL * S * V // 128  # 2048
    QN = BL * S // 128  # 32 rows per partition
    KA = 2  # ae batches per group

    with tile.TileContext(nc) as tc:
        with (
            tc.tile_pool(name="sb", bufs=5) as sb,
            tc.tile_pool(name="cst", bufs=1) as cst,
            tc.tile_pool(name="ps", bufs=1, space="PSUM") as ps,
        ):
            idx_t = cst.tile([SG, BL], I32)
            nc.sync.dma_start(idx_t[:], idx[:])

            # stats columns: 0=bias_sq 1=bias_mark_sq 2=kld_num 3=best_sq
            #                4=best_mark_sq; 16.. = per-piece ae_sq
            stats = cst.tile([128, NSTAT], F32)
            nc.vector.memset(stats[:], 0.0)
            acc_b = cst.tile([128, KB * P2], F32)
            nc.vector.memset(acc_b[:], 0.0)

            # --- KLD: sum q * (log(q + eps) - log(1/V)) via log(V*q + V*eps) ---
            qy_t = cst.tile([128, QCOLS], F32)
            nc.scalar.dma_start(
                qy_t[:].rearrange("p (n v) -> p n v", v=V),
                qy[:].rearrange("(p n) v -> p n v", n=QN),
            )
            lg = cst.tile([128, QCOLS], F32)
            ebias = cst.tile([128, 1], F32)
            nc.vector.memset(ebias[:], float(V) * EPS)
            nc.scalar.activation(lg[:], qy_t[:], ACTF.Ln, bias=ebias[:], scale=float(V))
            nc.vector.scalar_tensor_tensor(
                out=lg[:],
                in0=lg[:],
                scalar=0.0,
                in1=qy_t[:],
                op0=OP.subtract,
                op1=OP.mult,
                accum_out=stats[:, 2:3],
            )

            # --- BEST: full [B, P2] replicated on every core ---
            bt = sb.tile([128, P2], F32, tag="bt")
            nc.scalar.dma_start(bt[:], best[:])
            bgt = sb.tile([128, P2], F32, tag="bgt")
            nc.scalar.dma_start(bgt[:], best_gt[:])
            nc.vector.tensor_sub(bt[:], bt[:], bgt[:])
            nc.vector.tensor_mul(bt[:], bt[:], bt[:])
            nc.vector.reduce_sum(out=stats[:, 3:4], in_=bt[:], axis=AX.X)
            bm4 = cst.tile([128, 4], F32)
            for j, m in enumerate(MARK):
                nc.vector.reduce_sum(
                    out=bm4[:, j : j + 1], in_=bt[:, 2 * m : 2 * m + 2], axis=AX.X
                )
            nc.vector.reduce_sum(out=stats[:, 4:5], in_=bm4[:], axis=AX.X)

            # --- interleaved AE + BIAS groups ---
            # AE: sum (rzs[b, map[b,i]] - zs[b, i])^2, two batches per group.
            # BIAS: per-column accumulation of (pts_g - gts)^2, KB batches/group.
            zs_r = zs[:].rearrange("(b s) d -> s b d", s=S)
            gts_r = gts[:].rearrange("(b p) c -> p b c", p=SG)
            # AE pieces: 2-batch groups, then two singles for a short tail chain
            ae_pieces = [(g * KA, KA) for g in range(7)] + [(14, 1), (15, 1)]

            def bias_group(h):
                b0 = h * KB
                gt8 = sb.tile([128, KB * P2], F32, tag="gt8")
                nc.scalar.dma_start(
                    gt8[:].rearrange("p (k c) -> p k c", c=P2),
                    gts_r[:, b0 : b0 + KB, :],
                )
                pg8 = sb.tile([128, KB * P2], F32, tag="pg8")
                for k in range(KB):
                    nc.gpsimd.indirect_dma_start(
                        out=pg8[:, (k * P2) : ((k + 1) * P2)],
                        out_offset=None,
                        in_=pts[:],
                        in_offset=bass.IndirectOffsetOnAxis(
                            ap=idx_t[:, b0 + k : b0 + k + 1], axis=0
                        ),
                    )
                nc.vector.tensor_sub(pg8[:], pg8[:], gt8[:])
                nc.scalar.activation(pg8[:], pg8[:], ACTF.Square)
                nc.vector.tensor_add(acc_b[:], acc_b[:], pg8[:])

            def ae_piece(i):
                b0, ka = ae_pieces[i]
                rg = sb.tile([128, ka * D], F32, tag="rg")
                for k in range(ka):
                    nc.gpsimd.indirect_dma_start(
                        out=rg[:, (k * D) : ((k + 1) * D)],
                        out_offset=None,
                        in_=rzs[:],
                        in_offset=bass.IndirectOffsetOnAxis(
                            ap=idx_t[:, b0 + k : b0 + k + 1], axis=0
                        ),
                    )
                zt = sb.tile([128, ka * D], F32, tag="zt")
                nc.sync.dma_start(
                    zt[:].rearrange("p (k d) -> p k d", d=D),
                    zs_r[0:SG, b0 : b0 + ka, :],
                )
                nc.vector.tensor_sub(rg[:], rg[:], zt[:])
                nc.scalar.activation(
                    rg[:], rg[:], ACTF.Square,
                    accum_out=stats[:, AE0 + i : AE0 + i + 1],
                )

            bias_group(0)
            for i in range(5):
                ae_piece(i)
            bias_group(1)
            for i in range(5, len(ae_pieces)):
                ae_piece(i)

            # --- fold bias accumulator into stats ---
            nc.vector.reduce_sum(out=stats[:, 0:1], in_=acc_b[:], axis=AX.X)
            bk4 = cst.tile([128, 4], F32)
            acc_b3 = acc_b[:].rearrange("p (k c) -> p k c", c=P2)
            for j, m in enumerate(MARK):
                nc.vector.reduce_sum(
                    out=bk4[:, j : j + 1],
                    in_=acc_b3[:, :, 2 * m : 2 * m + 2],
                    axis=AX.XY,
                )
            nc.vector.reduce_sum(out=stats[:, 1:2], in_=bk4[:], axis=AX.X)

            # --- partition fold: ones^T @ stats -> [1, 8] ---
            ones = cst.tile([128, 1], F32)
            nc.vector.memset(ones[:], 1.0)
            pst = ps.tile([1, NSTAT], F32)
            nc.tensor.matmul(
                out=pst[:], lhsT=ones[:], rhs=stats[:], start=True, stop=True
            )
            res = cst.tile([1, NSTAT], F32)
            nc.vector.tensor_copy(res[:], pst[:])
            nc.sync.dma_start(out[:], res[:])

    nc.compile()
    return nc


def kernel(
    zs, rzs, pts, best, qy, gts, best_gt, mapping, vector_dims, **trace_kwargs
):
    global _module, last_results
    vd = int(np.asarray(vector_dims))
    assert vd == V, f"kernel compiled for vector_dims={V}, got {vd}"

    if _module is None:
        _module = _build_module()

    zs = np.asarray(zs, dtype=np.float32)
    rzs = np.asarray(rzs, dtype=np.float32)
    pts = np.asarray(pts, dtype=np.float32)
    gts = np.asarray(gts, dtype=np.float32)
    qy = np.asarray(qy, dtype=np.float32)
    mapping = np.asarray(mapping).astype(np.int32)
    best2 = np.ascontiguousarray(np.asarray(best, dtype=np.float32).reshape(B, P2))
    bgt2 = np.ascontiguousarray(np.asarray(best_gt, dtype=np.float32).reshape(B, P2))

    base = (np.arange(BL, dtype=np.int32) * S)[:, None]
    in_maps = []
    for c in range(NCORES):
        sl = slice(c * BL, (c + 1) * BL)
        in_maps.append(
            {
                "zs": zs[sl].reshape(BL * S, D),
                "rzs": rzs[sl].reshape(BL * S, D),
                "pts": pts[sl].reshape(BL * S, P2),
                "gts": gts[sl].reshape(BL * SG, P2),
                "qy": qy[sl].reshape(BL * S, V),
                "best": best2,
                "best_gt": bgt2,
                "idx": np.ascontiguousarray((mapping[sl] + base).T),
            }
        )

    last_results = run_bass_kernel_spmd(
        _module, in_maps, list(range(NCORES)), **trace_kwargs
    )
    parts = np.stack(
        [
            np.asarray(r["out"], dtype=np.float64).reshape(NSTAT)
            for r in last_results.results
        ]
    )
    tot = parts.sum(axis=0)

    ae_loss = tot[AE0:].sum() / (B * SG * D)
    bias_loss = tot[0] / (B * SG * P2) + ALPHA * tot[1] / (B * SG * 2 * len(MARK))
    kld_loss = tot[2] / (B * S)
    best_mse = parts[0][3] / (B * P2) + ALPHA * parts[0][4] / (B * 2 * len(MARK))

    return np.array(kld_loss + ae_loss + best_mse + bias_loss, dtype=np.float32)
